# revision 1
# baseline (speedup 1.0000x reference)
"""Trainium2 Bass kernel for nn_MetricalGNN (2-layer hetero GraphSAGE).

Math (per layer, T=4 edge types):
    out = h @ mean_t(W_self[t]) + mean_t(b[t])
        + (1/T) * sum_t diag(1/max(cnt_t,1)) @ segsum_t(h[src]) @ W_neigh[t]
Layer 1 is followed by row-wise L2 normalize + ReLU.

Device strategy (8 cores, destination-sharded):
  - Each core owns a contiguous 6250-node destination range, processed in
    49 windows of 128 destinations.
  - Edges are sorted by (core, window, type) on the host and packed into
    128-edge chunks; all cores share one static chunk schedule (max over
    cores per (window, type) slot, padded).
  - Per chunk: one slice of a batched indirect DMA gathers the 128 source
    rows (fp16); DVE builds a scaled one-hot A[e,d] = scale_e * (iota==dst_e)
    in a single tensor_scalar op; the TensorEngine accumulates
    S_t^T[f,d] += M^T A into a per-type PSUM bank.  The per-edge scale folds
    in the 1/cnt mean, the 1/T type average, and padding (dst=200 -> 0 row).
  - The self term h_w @ W_self_avg is computed by the same machinery as a
    5th "type" whose edges are the window's own nodes with scale 1: its
    S^T is then exactly h_w^T, needing no separate transpose.
  - Stage 2 per window: copy the five S^T banks to SBUF (cast fp16) and run
    five matmuls out[d,fo] += S_t @ W_t into one PSUM bank, plus a K=1
    ones-row matmul adding the mean bias.
  - Layer-1 epilogue fuses square+row-sum (ACT accum), sqrt, reciprocal,
    and relu-with-per-row-scale (ACT) before storing h1 as fp16.
  - An AllGather exchanges the per-core h1 slices between layers; layer 2
    gathers from the concatenated [8*6272, 128] buffer via remapped indices.

Inputs are replicated/sharded on the host: x is pre-cast to fp16 and
replicated; per-core metadata tensors carry gather indices, window-local
destinations and scales; weights are packed to fp16 once.
"""

import numpy as np

N = 50000
E = 600000
F = 128
T = 4
C = 8                      # cores
NPC = N // C               # 6250 destinations per core
WPC = (NPC + 127) // 128   # 49 windows per core
NPC_PAD = WPC * 128        # 6272 rows per core slice
GB = 3                     # windows per batched gather instruction
PAD_DST = 200.0            # one-hot miss -> zero column


def _prep(x, W_self1, W_neigh1, b1, W_self2, W_neigh2, b2, edge_index, edge_type):
    src = np.asarray(edge_index[0], dtype=np.int64)
    dst = np.asarray(edge_index[1], dtype=np.int64)
    et = np.asarray(edge_type, dtype=np.int64)

    cnt = np.bincount(et * N + dst, minlength=T * N).reshape(T, N).astype(np.float32)
    scale_e = (0.25 / np.maximum(cnt[et, dst], 1.0)).astype(np.float32)

    core = dst // NPC
    win = (dst % NPC) // 128
    dloc = ((dst % NPC) % 128).astype(np.float32)

    order = np.lexsort((et, win, core))
    src_s, et_s, core_s, win_s = src[order], et[order], core[order], win[order]
    dloc_s, scale_s = dloc[order], scale_e[order]

    gkey = (core_s * WPC + win_s) * T + et_s
    counts = np.bincount(gkey, minlength=C * WPC * T).reshape(C, WPC, T)
    nchunk = np.maximum(1, -(-counts.max(axis=0) // 128))  # [WPC, T]

    # chunk layout: for w: [t0 chunks..., t1..., t2..., t3..., self]
    chunks_per_win = nchunk.sum(axis=1) + 1
    win_chunk_base = np.zeros(WPC, dtype=np.int64)
    win_chunk_base[1:] = np.cumsum(chunks_per_win)[:-1]
    NCH = int(chunks_per_win.sum())

    idx1 = np.zeros((C, NCH, 128), dtype=np.int32)
    idx2 = np.zeros((C, NCH, 128), dtype=np.int32)
    dstc = np.full((C, NCH, 128), PAD_DST, dtype=np.float32)
    sclc = np.zeros((C, NCH, 128), dtype=np.float32)

    glo = np.zeros(C * WPC * T + 1, dtype=np.int64)
    np.cumsum(np.bincount(gkey, minlength=C * WPC * T), out=glo[1:])

    permpos = (src // NPC) * NPC_PAD + (src % NPC)
    permpos_s = permpos[order].astype(np.int32)
    src_s32 = src_s.astype(np.int32)

    for c in range(C):
        for w in range(WPC):
            base = win_chunk_base[w]
            toff = 0
            for t in range(T):
                g = (c * WPC + w) * T + t
                lo, hi = glo[g], glo[g + 1]
                n = hi - lo
                s0 = (base + toff) * 128
                flat_i1 = idx1[c].reshape(-1)
                flat_i2 = idx2[c].reshape(-1)
                flat_d = dstc[c].reshape(-1)
                flat_s = sclc[c].reshape(-1)
                flat_i1[s0:s0 + n] = src_s32[lo:hi]
                flat_i2[s0:s0 + n] = permpos_s[lo:hi]
                flat_d[s0:s0 + n] = dloc_s[lo:hi]
                flat_s[s0:s0 + n] = scale_s[lo:hi]
                toff += nchunk[w, t]
            # self chunk
            sc = base + toff
            nd = min(128, NPC - w * 128)
            nodes = c * NPC + w * 128 + np.arange(nd)
            idx1[c, sc, :nd] = nodes.astype(np.int32)
            idx2[c, sc, :nd] = (c * NPC_PAD + w * 128 + np.arange(nd)).astype(np.int32)
            dstc[c, sc, :nd] = np.arange(nd, dtype=np.float32)
            sclc[c, sc, :nd] = 1.0

    # [C, NCH, 128] -> [C, 128, NCH] so column k holds chunk k's 128 rows
    idx1 = np.ascontiguousarray(idx1.transpose(0, 2, 1))
    idx2 = np.ascontiguousarray(idx2.transpose(0, 2, 1))
    dstc = np.ascontiguousarray(dstc.transpose(0, 2, 1))
    sclc = np.ascontiguousarray(sclc.transpose(0, 2, 1))

    wpack = np.empty((2 * (T + 1), F, F), dtype=np.float16)
    wpack[0:T] = np.asarray(W_neigh1, np.float32).astype(np.float16)
    wpack[T] = np.asarray(W_self1, np.float32).mean(axis=0).astype(np.float16)
    wpack[T + 1:2 * T + 1] = np.asarray(W_neigh2, np.float32).astype(np.float16)
    wpack[2 * T + 1] = np.asarray(W_self2, np.float32).mean(axis=0).astype(np.float16)

    bpack = np.stack([
        np.asarray(b1, np.float32).mean(axis=0),
        np.asarray(b2, np.float32).mean(axis=0),
    ]).astype(np.float16)

    x16 = np.asarray(x, np.float32).astype(np.float16)
    x16my = np.zeros((C, NPC_PAD, F), dtype=np.float16)
    for c in range(C):
        x16my[c, :NPC] = x16[c * NPC:(c + 1) * NPC]
    return idx1, idx2, dstc, sclc, wpack, bpack, x16, x16my, nchunk, NCH


def _legalize_sync_waits(nc, max_waits=1):
    """The walrus build in this container caps sync-wait commands per
    instruction; hoist excess waits onto NOPs inserted before the
    instruction on the same engine (sequencers execute in order)."""
    from concourse import mybir

    ctr = [0]
    for fn in nc.m.functions:
        for bb in fn.blocks:
            insts = bb.instructions
            if not any(
                i.sync_info is not None and len(i.sync_info.on_wait) > max_waits
                for i in insts
            ):
                continue
            out = []
            for inst in insts:
                si = inst.sync_info
                if si is not None and len(si.on_wait) > max_waits:
                    waits = list(si.on_wait)
                    keep = waits[-max_waits:]
                    hoist = waits[:-max_waits]
                    for i in range(0, len(hoist), max_waits):
                        nop = mybir.InstNoOp(
                            name=f"I-waitsplit-{ctr[0]}", ins=[], outs=[])
                        ctr[0] += 1
                        nop.engine = inst.engine
                        nop.sync_info = mybir.SyncInfo(
                            on_wait=hoist[i:i + max_waits], on_update=[])
                        out.append(nop)
                    inst.sync_info = mybir.SyncInfo(
                        on_wait=keep, on_update=list(si.on_update))
                out.append(inst)
            insts.clear()
            insts.extend(out)


def build_module(NCH, nchunk, legalize=True, n_cores=C):
    import concourse.bass as bass
    import concourse.tile as tile
    from concourse import mybir

    f16, f32, i32 = mybir.dt.float16, mybir.dt.float32, mybir.dt.int32
    Alu = mybir.AluOpType
    Act = mybir.ActivationFunctionType

    nc = bass.Bass(trn_type="TRN2")
    t_x16 = nc.dram_tensor("x16", [N, F], f16, kind="ExternalInput")
    t_x16my = nc.dram_tensor("x16my", [NPC_PAD, F], f16, kind="ExternalInput")
    t_idx1 = nc.dram_tensor("idx1", [128, NCH], i32, kind="ExternalInput")
    t_idx2 = nc.dram_tensor("idx2", [128, NCH], i32, kind="ExternalInput")
    t_dstc = nc.dram_tensor("dstc", [128, NCH], f32, kind="ExternalInput")
    t_sclc = nc.dram_tensor("sclc", [128, NCH], f32, kind="ExternalInput")
    t_wpack = nc.dram_tensor("wpack", [2 * (T + 1), F, F], f16, kind="ExternalInput")
    t_bpack = nc.dram_tensor("bpack", [2, F], f16, kind="ExternalInput")
    t_out = nc.dram_tensor("out", [NPC_PAD, F], f32, kind="ExternalOutput")

    chunks_per_win = nchunk.sum(axis=1) + 1
    win_chunk_base = np.zeros(WPC, dtype=np.int64)
    win_chunk_base[1:] = np.cumsum(chunks_per_win)[:-1]

    with tile.TileContext(nc, num_cores=n_cores) as tc:
        with tc.tile_pool(name="const", bufs=1) as cpool, \
             tc.tile_pool(name="gath", bufs=2) as gpool, \
             tc.tile_pool(name="onehot", bufs=6) as apool, \
             tc.tile_pool(name="stage2", bufs=2) as spool, \
             tc.tile_pool(name="epi", bufs=2) as epool, \
             tc.tile_pool(name="spsum", bufs=1, space="PSUM") as pspool, \
             tc.tile_pool(name="opsum", bufs=2, space="PSUM") as opool, \
             tc.tile_pool(name="dram", bufs=1, space="DRAM") as dpool:

            idx1_t = cpool.tile([128, NCH], i32)
            nc.sync.dma_start(out=idx1_t[:], in_=t_idx1[:])
            idx2_t = cpool.tile([128, NCH], i32)
            nc.sync.dma_start(out=idx2_t[:], in_=t_idx2[:])
            dstc_t = cpool.tile([128, NCH], f32)
            nc.sync.dma_start(out=dstc_t[:], in_=t_dstc[:])
            sclc_t = cpool.tile([128, NCH], f32)
            nc.sync.dma_start(out=sclc_t[:], in_=t_sclc[:])

            w_sb = cpool.tile([128, 2 * (T + 1) * F], f16)
            for k in range(2 * (T + 1)):
                nc.sync.dma_start(out=w_sb[:, k * F:(k + 1) * F], in_=t_wpack[k])
            b_sb = cpool.tile([1, 2 * F], f16)
            nc.sync.dma_start(out=b_sb[:, :F], in_=t_bpack[0:1, :])
            nc.sync.dma_start(out=b_sb[:, F:], in_=t_bpack[1:2, :])
            ones_sb = cpool.tile([1, 128], f16)
            nc.vector.memset(ones_sb[:], 1.0)
            eps_sb = cpool.tile([128, 1], f32)
            nc.vector.memset(eps_sb[:], 1e-24)
            zero_sb = cpool.tile([128, 1], f32)
            nc.vector.memset(zero_sb[:], 0.0)

            iota_i = cpool.tile([128, 128], i32)
            nc.gpsimd.iota(iota_i[:], pattern=[[1, 128]], base=0, channel_multiplier=0)
            iota_t = cpool.tile([128, 128], f32)
            nc.vector.tensor_copy(out=iota_t[:], in_=iota_i[:])

            h1_my = dpool.tile([NPC_PAD, F], f16)
            h1_all = dpool.tile([C * NPC_PAD, F], f16, addr_space="Shared")

            for layer in (0, 1):
                src_tbl = t_x16 if layer == 0 else h1_all
                self_tbl = t_x16my if layer == 0 else h1_my
                idx_t = idx1_t if layer == 0 else idx2_t
                wofs = layer * (T + 1) * F

                ss_all = epool.tile([128, WPC], f32, name=f"ss_all{layer}",
                                    tag=f"ss_all{layer}", bufs=1)
                o16 = []

                for w in range(WPC):
                    base = int(win_chunk_base[w])
                    s_ps = [pspool.tile([128, 128], f32, space="PSUM",
                                        name=f"s{t}", tag=f"s{t}")
                            for t in range(T + 1)]
                    ch = base
                    for t in range(T):
                        nk = int(nchunk[w, t])
                        for k in range(nk):
                            m_t = gpool.tile([128, F], f16, tag="m")
                            nc.gpsimd.indirect_dma_start(
                                out=m_t[:], out_offset=None, in_=src_tbl[:],
                                in_offset=bass.IndirectOffsetOnAxis(
                                    ap=idx_t[:, ch:ch + 1], axis=0))
                            a_t = apool.tile([128, 128], f16, tag="a")
                            nc.vector.tensor_scalar(
                                out=a_t[:], in0=iota_t[:],
                                scalar1=dstc_t[:, ch:ch + 1],
                                scalar2=sclc_t[:, ch:ch + 1],
                                op0=Alu.is_equal, op1=Alu.mult)
                            nc.tensor.matmul(
                                out=s_ps[t][:], lhsT=m_t[:], rhs=a_t[:],
                                start=(k == 0), stop=(k == nk - 1))
                            ch += 1
                    # self chunk: contiguous rows of my own slice
                    m_t = gpool.tile([128, F], f16, tag="m")
                    nc.sync.dma_start(
                        out=m_t[:], in_=self_tbl[w * 128:(w + 1) * 128, :])
                    a_t = apool.tile([128, 128], f16, tag="a")
                    nc.vector.tensor_scalar(
                        out=a_t[:], in0=iota_t[:],
                        scalar1=dstc_t[:, ch:ch + 1],
                        scalar2=sclc_t[:, ch:ch + 1],
                        op0=Alu.is_equal, op1=Alu.mult)
                    nc.tensor.matmul(
                        out=s_ps[T][:], lhsT=m_t[:], rhs=a_t[:],
                        start=True, stop=True)
                    ch += 1

                    # stage 2
                    o_ps = opool.tile([128, 128], f32, space="PSUM", tag="o")
                    s_sb = []
                    for t in range(T + 1):
                        st = spool.tile([128, 128], f16, tag=f"ssb{t}",
                                        name=f"ssb{t}")
                        if t < 3:
                            nc.vector.tensor_copy(out=st[:], in_=s_ps[t][:])
                        else:
                            nc.scalar.activation(out=st[:], in_=s_ps[t][:],
                                                 func=Act.Copy)
                        s_sb.append(st)
                    for t in range(T + 1):
                        nc.tensor.matmul(
                            out=o_ps[:], lhsT=s_sb[t][:],
                            rhs=w_sb[:, wofs + t * F: wofs + (t + 1) * F],
                            start=(t == 0), stop=False)
                    nc.tensor.matmul(
                        out=o_ps[:], lhsT=ones_sb[:],
                        rhs=b_sb[:, layer * F:(layer + 1) * F],
                        start=False, stop=True)

                    if layer == 0:
                        # stage to fp16 SBUF; square+reduce row sums now,
                        # sqrt/reciprocal batched once per layer
                        ow = epool.tile([128, 128], f16, name=f"o16_{w}",
                                        tag=f"o16_{w}", bufs=1)
                        nc.scalar.activation(out=ow[:], in_=o_ps[:],
                                             func=Act.Copy)
                        o16.append(ow)
                        sq = epool.tile([128, 128], f16, tag="sq")
                        nc.vector.tensor_tensor(
                            out=sq[:], in0=ow[:], in1=ow[:], op=Alu.mult)
                        nc.vector.tensor_reduce(
                            out=ss_all[:, w:w + 1], in_=sq[:],
                            axis=mybir.AxisListType.X, op=Alu.add)
                    else:
                        o_sb = epool.tile([128, 128], f32, tag="osb")
                        nc.scalar.activation(out=o_sb[:], in_=o_ps[:],
                                             func=Act.Copy)
                        nc.sync.dma_start(
                            out=t_out[w * 128:(w + 1) * 128, :], in_=o_sb[:])

                if layer == 0:
                    nrm_all = epool.tile([128, WPC], f32, name="nrm_all",
                                         tag="nrm_all", bufs=1)
                    nc.scalar.activation(out=nrm_all[:], in_=ss_all[:],
                                         func=Act.Sqrt, bias=eps_sb[:])
                    rn_all = epool.tile([128, WPC], f32, name="rn_all",
                                        tag="rn_all", bufs=1)
                    nc.vector.reciprocal(out=rn_all[:], in_=nrm_all[:])
                    for w in range(WPC):
                        h1_sb = epool.tile([128, 128], f16, tag="h1")
                        nc.vector.tensor_scalar(
                            out=h1_sb[:], in0=o16[w][:],
                            scalar1=rn_all[:, w:w + 1],
                            scalar2=zero_sb[:],
                            op0=Alu.mult, op1=Alu.max)
                        nc.sync.dma_start(
                            out=h1_my[w * 128:(w + 1) * 128, :], in_=h1_sb[:])
                    nc.gpsimd.collective_compute(
                        "AllGather",
                        mybir.AluOpType.bypass,
                        replica_groups=[list(range(n_cores))],
                        ins=[h1_my.opt()],
                        outs=[h1_all.opt()],
                    )

    if legalize:
        _legalize_sync_waits(nc)
    return nc


def kernel(**inputs):
    import sys
    if '/opt/trn_rl_repo' not in sys.path:
        sys.path.insert(0, '/opt/trn_rl_repo')

    idx1, idx2, dstc, sclc, wpack, bpack, x16, x16my, nchunk, NCH = _prep(
        inputs["x"], inputs["W_self1"], inputs["W_neigh1"], inputs["b1"],
        inputs["W_self2"], inputs["W_neigh2"], inputs["b2"],
        inputs["edge_index"], inputs["edge_type"])

    nc = build_module(NCH, nchunk, legalize=True, n_cores=C)

    from concourse.bass_utils import run_bass_kernel_spmd
    in_maps = [
        {"x16": x16, "x16my": x16my[c], "idx1": idx1[c], "idx2": idx2[c],
         "dstc": dstc[c], "sclc": sclc[c], "wpack": wpack, "bpack": bpack}
        for c in range(C)
    ]
    res = run_bass_kernel_spmd(nc, in_maps, core_ids=list(range(C)))

    out = np.empty((N, F), dtype=np.float32)
    for c in range(C):
        out[c * NPC:(c + 1) * NPC] = res.results[c]["out"][:NPC]
    return out



# revision 4
# speedup vs baseline: 3.2727x; 3.2727x over previous
"""Trainium2 Bass kernel for nn_MetricalGNN (2-layer hetero GraphSAGE).

Math (per layer, T=4 edge types):
    out = h @ mean_t(W_self[t]) + mean_t(b[t])
        + (1/T) * sum_t diag(1/max(cnt_t,1)) @ segsum_t(h[src]) @ W_neigh[t]
Layer 1 is followed by row-wise L2 normalize + ReLU.

Device strategy (8 cores, destination-sharded):
  - Each core owns a contiguous 6250-node destination range, processed in
    49 windows of 128 destinations.
  - Edges are sorted by (core, window, type) on the host and packed into
    128-edge chunks; all cores share one static chunk schedule (max over
    cores per (window, type) slot, padded).  A per-window "self" chunk
    (the window's own 128 nodes, scale 1) rides along as a 5th type.
  - All chunks of a window PAIR are fetched by ONE batched indirect DMA
    (SWDGE desc-gen has ~1us fixed cost per instruction, so per-chunk
    gathers are Pool-bound; batching amortizes it ~34x).
  - Per chunk: DVE builds a scaled one-hot A[e,d] = scale_e*(iota==dst_e)
    in one fp16 tensor_scalar (2x DVE mode); the TensorEngine accumulates
    S_t^T[f,d] += M^T A into a per-type PSUM bank.  The per-edge scale
    folds in the 1/cnt mean, the 1/T type average, and padding
    (dst=200 -> zero column).  Self chunks of full windows use a shared
    constant identity matrix instead (no per-window DVE op).
  - Stage 2 per window: copy the five S^T banks to SBUF (fp16) and run
    five matmuls out[d,fo] += S_t @ W_t into one PSUM bank, plus a K=1
    ones-row matmul adding the mean bias.
  - Layer-1 epilogue: ACT Square+accum produces row sums of squares
    directly from PSUM; sqrt/reciprocal batched once per layer; DVE
    applies relu(h*rn) and h1 is stored as fp16.
  - An AllGather exchanges the per-core h1 slices between layers; layer 2
    gathers from the concatenated [8*6272, 128] buffer via remapped
    indices.

Inputs are replicated/sharded on the host: x is pre-cast to fp16 and
replicated; per-core metadata tensors carry gather indices and
window-local destinations/scales (fp16); weights are packed to fp16 once.
"""

import numpy as np

N = 50000
E = 600000
F = 128
T = 4
C = 8                      # cores
NPC = N // C               # 6250 destinations per core
WPC = (NPC + 127) // 128   # 49 windows per core
NPC_PAD = WPC * 128        # 6272 rows per core slice
GB = 2                     # windows per batched gather instruction
PAD_DST = 200.0            # one-hot miss -> zero column


def _win_groups():
    """Windows grouped GB at a time for batched gathers."""
    return [tuple(range(w, min(w + GB, WPC))) for w in range(0, WPC, GB)]


def _prep(x, W_self1, W_neigh1, b1, W_self2, W_neigh2, b2, edge_index, edge_type):
    src = np.asarray(edge_index[0], dtype=np.int64)
    dst = np.asarray(edge_index[1], dtype=np.int64)
    et = np.asarray(edge_type, dtype=np.int64)

    cnt = np.bincount(et * N + dst, minlength=T * N).reshape(T, N).astype(np.float32)
    scale_e = (0.25 / np.maximum(cnt[et, dst], 1.0)).astype(np.float32)

    core = dst // NPC
    win = (dst % NPC) // 128
    dloc = ((dst % NPC) % 128).astype(np.float32)

    order = np.lexsort((et, win, core))
    src_s, et_s, core_s, win_s = src[order], et[order], core[order], win[order]
    dloc_s, scale_s = dloc[order], scale_e[order]

    gkey = (core_s * WPC + win_s) * T + et_s
    counts = np.bincount(gkey, minlength=C * WPC * T).reshape(C, WPC, T)
    nchunk = np.maximum(1, -(-counts.max(axis=0) // 128))  # [WPC, T]

    # chunk layout: for w: [t0 chunks..., t1..., t2..., t3..., self]
    chunks_per_win = nchunk.sum(axis=1) + 1
    win_chunk_base = np.zeros(WPC, dtype=np.int64)
    win_chunk_base[1:] = np.cumsum(chunks_per_win)[:-1]
    NCH = int(chunks_per_win.sum())

    idx1 = np.zeros((C, NCH, 128), dtype=np.int32)
    idx2 = np.zeros((C, NCH, 128), dtype=np.int32)
    dstc = np.full((C, NCH, 128), PAD_DST, dtype=np.float32)
    sclc = np.zeros((C, NCH, 128), dtype=np.float32)

    glo = np.zeros(C * WPC * T + 1, dtype=np.int64)
    np.cumsum(np.bincount(gkey, minlength=C * WPC * T), out=glo[1:])

    permpos = (src // NPC) * NPC_PAD + (src % NPC)
    permpos_s = permpos[order].astype(np.int32)
    src_s32 = src_s.astype(np.int32)

    for c in range(C):
        flat_i1 = idx1[c].reshape(-1)
        flat_i2 = idx2[c].reshape(-1)
        flat_d = dstc[c].reshape(-1)
        flat_s = sclc[c].reshape(-1)
        for w in range(WPC):
            base = win_chunk_base[w]
            toff = 0
            for t in range(T):
                g = (c * WPC + w) * T + t
                lo, hi = glo[g], glo[g + 1]
                n = hi - lo
                s0 = (base + toff) * 128
                flat_i1[s0:s0 + n] = src_s32[lo:hi]
                flat_i2[s0:s0 + n] = permpos_s[lo:hi]
                flat_d[s0:s0 + n] = dloc_s[lo:hi]
                flat_s[s0:s0 + n] = scale_s[lo:hi]
                toff += nchunk[w, t]
            # self chunk
            sc = base + toff
            nd = min(128, NPC - w * 128)
            nodes = c * NPC + w * 128 + np.arange(nd)
            idx1[c, sc, :nd] = nodes.astype(np.int32)
            idx2[c, sc, :nd] = (c * NPC_PAD + w * 128 + np.arange(nd)).astype(np.int32)
            dstc[c, sc, :nd] = np.arange(nd, dtype=np.float32)
            sclc[c, sc, :nd] = 1.0

    # [C, NCH, 128] -> [C, 128, NCH] so column k holds chunk k's 128 rows
    idx1 = np.ascontiguousarray(idx1.transpose(0, 2, 1))
    idx2 = np.ascontiguousarray(idx2.transpose(0, 2, 1))
    dstc = np.ascontiguousarray(dstc.transpose(0, 2, 1))
    sclc = np.ascontiguousarray(sclc.transpose(0, 2, 1))

    wpack = np.empty((2 * (T + 1), F, F), dtype=np.float16)
    wpack[0:T] = np.asarray(W_neigh1, np.float32).astype(np.float16)
    wpack[T] = np.asarray(W_self1, np.float32).mean(axis=0).astype(np.float16)
    wpack[T + 1:2 * T + 1] = np.asarray(W_neigh2, np.float32).astype(np.float16)
    wpack[2 * T + 1] = np.asarray(W_self2, np.float32).mean(axis=0).astype(np.float16)

    bpack = np.stack([
        np.asarray(b1, np.float32).mean(axis=0),
        np.asarray(b2, np.float32).mean(axis=0),
    ]).astype(np.float16)

    x16 = np.asarray(x, np.float32).astype(np.float16)
    return idx1, idx2, dstc, sclc, wpack, bpack, x16, nchunk, NCH


def make_in_maps(prep):
    idx1, idx2, dstc, sclc, wpack, bpack, x16, nchunk, NCH = prep
    return [
        {"x16": x16, "idx1": idx1[c], "idx2": idx2[c],
         "dstc": dstc[c], "sclc": sclc[c], "wpack": wpack, "bpack": bpack}
        for c in range(C)
    ]


def _legalize_sync_waits(nc, max_waits=1):
    """The walrus build in this container caps sync-wait commands per
    instruction; hoist excess waits onto NOPs inserted before the
    instruction on the same engine (sequencers execute in order)."""
    from concourse import mybir

    ctr = [0]
    for fn in nc.m.functions:
        for bb in fn.blocks:
            insts = bb.instructions
            if not any(
                i.sync_info is not None and len(i.sync_info.on_wait) > max_waits
                for i in insts
            ):
                continue
            out = []
            for inst in insts:
                si = inst.sync_info
                if si is not None and len(si.on_wait) > max_waits:
                    waits = list(si.on_wait)
                    keep = waits[-max_waits:]
                    hoist = waits[:-max_waits]
                    for i in range(0, len(hoist), max_waits):
                        nop = mybir.InstNoOp(
                            name=f"I-waitsplit-{ctr[0]}", ins=[], outs=[])
                        ctr[0] += 1
                        nop.engine = inst.engine
                        nop.sync_info = mybir.SyncInfo(
                            on_wait=hoist[i:i + max_waits], on_update=[])
                        out.append(nop)
                    inst.sync_info = mybir.SyncInfo(
                        on_wait=keep, on_update=list(si.on_update))
                out.append(inst)
            insts.clear()
            insts.extend(out)


def build_module(NCH, nchunk, legalize=True, n_cores=C):
    import concourse.bass as bass
    import concourse.tile as tile
    from concourse import mybir

    f16, f32, i32 = mybir.dt.float16, mybir.dt.float32, mybir.dt.int32
    Alu = mybir.AluOpType
    Act = mybir.ActivationFunctionType

    nc = bass.Bass(trn_type="TRN2")
    t_x16 = nc.dram_tensor("x16", [N, F], f16, kind="ExternalInput")
    t_idx1 = nc.dram_tensor("idx1", [128, NCH], i32, kind="ExternalInput")
    t_idx2 = nc.dram_tensor("idx2", [128, NCH], i32, kind="ExternalInput")
    t_dstc = nc.dram_tensor("dstc", [128, NCH], f32, kind="ExternalInput")
    t_sclc = nc.dram_tensor("sclc", [128, NCH], f32, kind="ExternalInput")
    t_wpack = nc.dram_tensor("wpack", [2 * (T + 1), F, F], f16, kind="ExternalInput")
    t_bpack = nc.dram_tensor("bpack", [2, F], f16, kind="ExternalInput")
    t_out = nc.dram_tensor("out", [NPC_PAD, F], f32, kind="ExternalOutput")

    chunks_per_win = nchunk.sum(axis=1) + 1
    win_chunk_base = np.zeros(WPC, dtype=np.int64)
    win_chunk_base[1:] = np.cumsum(chunks_per_win)[:-1]
    groups = _win_groups()
    maxcols = max(int(sum(chunks_per_win[w] for w in g)) for g in groups)

    with tile.TileContext(nc, num_cores=n_cores) as tc:
        with tc.tile_pool(name="const", bufs=1) as cpool, \
             tc.tile_pool(name="gath", bufs=2) as gpool, \
             tc.tile_pool(name="onehot", bufs=8) as apool, \
             tc.tile_pool(name="stage2", bufs=2) as spool, \
             tc.tile_pool(name="epi", bufs=2) as epool, \
             tc.tile_pool(name="spsum", bufs=1, space="PSUM") as pspool, \
             tc.tile_pool(name="opsum", bufs=2, space="PSUM") as opool, \
             tc.tile_pool(name="dram", bufs=1, space="DRAM") as dpool:

            idx1_t = cpool.tile([128, NCH], i32)
            nc.sync.dma_start(out=idx1_t[:], in_=t_idx1[:])
            idx2_t = cpool.tile([128, NCH], i32)
            nc.sync.dma_start(out=idx2_t[:], in_=t_idx2[:])
            dstc_t = cpool.tile([128, NCH], f32)
            nc.sync.dma_start(out=dstc_t[:], in_=t_dstc[:])
            sclc_t = cpool.tile([128, NCH], f32)
            nc.sync.dma_start(out=sclc_t[:], in_=t_sclc[:])

            w_sb = cpool.tile([128, 2 * (T + 1) * F], f16)
            for k in range(2 * (T + 1)):
                nc.sync.dma_start(out=w_sb[:, k * F:(k + 1) * F], in_=t_wpack[k])
            b_sb = cpool.tile([1, 2 * F], f16)
            nc.sync.dma_start(out=b_sb[:, :F], in_=t_bpack[0:1, :])
            nc.sync.dma_start(out=b_sb[:, F:], in_=t_bpack[1:2, :])
            ones_sb = cpool.tile([1, 128], f16)
            nc.vector.memset(ones_sb[:], 1.0)
            eps_sb = cpool.tile([128, 1], f32)
            nc.vector.memset(eps_sb[:], 1e-24)
            zero_sb = cpool.tile([128, 1], f32)
            nc.vector.memset(zero_sb[:], 0.0)

            iota_i = cpool.tile([128, 128], i32)
            nc.gpsimd.iota(iota_i[:], pattern=[[1, 128]], base=0, channel_multiplier=0)
            iota16 = cpool.tile([128, 128], f16)
            nc.vector.tensor_copy(out=iota16[:], in_=iota_i[:])
            iotap_i = cpool.tile([128, 1], i32)
            nc.gpsimd.iota(iotap_i[:], pattern=[[0, 1]], base=0, channel_multiplier=1)
            iotap32 = cpool.tile([128, 1], f32)
            nc.vector.tensor_copy(out=iotap32[:], in_=iotap_i[:])
            # shared identity one-hot for full self chunks
            ident = cpool.tile([128, 128], f16)
            nc.vector.tensor_scalar(
                out=ident[:], in0=iota16[:], scalar1=iotap32[:],
                scalar2=None, op0=Alu.is_equal)

            h1_my = dpool.tile([NPC_PAD, F], f16)
            h1_all = dpool.tile([C * NPC_PAD, F], f16, addr_space="Shared")

            for layer in (0, 1):
                src_tbl = t_x16 if layer == 0 else h1_all
                idx_t = idx1_t if layer == 0 else idx2_t
                wofs = layer * (T + 1) * F

                ss_all = epool.tile([128, WPC], f32, name=f"ss_all{layer}",
                                    tag=f"ss_all{layer}", bufs=1)
                o16 = []

                for grp in groups:
                    col0 = int(win_chunk_base[grp[0]])
                    cols = int(sum(chunks_per_win[w] for w in grp))
                    m_t = gpool.tile([128, maxcols * F], f16, tag="m")
                    nc.gpsimd.indirect_dma_start(
                        out=m_t[:, :cols * F], out_offset=None, in_=src_tbl[:],
                        in_offset=bass.IndirectOffsetOnAxis(
                            ap=idx_t[:, col0:col0 + cols], axis=0))

                    for w in grp:
                        s_ps = [pspool.tile([128, 128], f32, space="PSUM",
                                            name=f"s{t}", tag=f"s{t}")
                                for t in range(T + 1)]
                        ch = int(win_chunk_base[w])
                        for t in range(T):
                            nk = int(nchunk[w, t])
                            for k in range(nk):
                                a_t = apool.tile([128, 128], f16, tag="a")
                                nc.vector.tensor_scalar(
                                    out=a_t[:], in0=iota16[:],
                                    scalar1=dstc_t[:, ch:ch + 1],
                                    scalar2=sclc_t[:, ch:ch + 1],
                                    op0=Alu.is_equal, op1=Alu.mult)
                                off = (ch - col0) * F
                                nc.tensor.matmul(
                                    out=s_ps[t][:], lhsT=m_t[:, off:off + F],
                                    rhs=a_t[:], start=(k == 0), stop=(k == nk - 1))
                                ch += 1
                        # self chunk: identity one-hot (partial last window
                        # builds its own to zero the tail rows)
                        off = (ch - col0) * F
                        if w == WPC - 1:
                            a_t = apool.tile([128, 128], f16, tag="a")
                            nc.vector.tensor_scalar(
                                out=a_t[:], in0=iota16[:],
                                scalar1=dstc_t[:, ch:ch + 1],
                                scalar2=sclc_t[:, ch:ch + 1],
                                op0=Alu.is_equal, op1=Alu.mult)
                            rhs_self = a_t
                        else:
                            rhs_self = ident
                        nc.tensor.matmul(
                            out=s_ps[T][:], lhsT=m_t[:, off:off + F],
                            rhs=rhs_self[:], start=True, stop=True)
                        ch += 1

                        # stage 2
                        o_ps = opool.tile([128, 128], f32, space="PSUM", tag="o")
                        s_sb = []
                        for t in range(T + 1):
                            st = spool.tile([128, 128], f16, tag=f"ssb{t}",
                                            name=f"ssb{t}")
                            if t < 2:
                                nc.vector.tensor_copy(out=st[:], in_=s_ps[t][:])
                            else:
                                nc.scalar.activation(out=st[:], in_=s_ps[t][:],
                                                     func=Act.Copy)
                            s_sb.append(st)
                        for t in range(T + 1):
                            nc.tensor.matmul(
                                out=o_ps[:], lhsT=s_sb[t][:],
                                rhs=w_sb[:, wofs + t * F: wofs + (t + 1) * F],
                                start=(t == 0), stop=False)
                        nc.tensor.matmul(
                            out=o_ps[:], lhsT=ones_sb[:],
                            rhs=b_sb[:, layer * F:(layer + 1) * F],
                            start=False, stop=True)

                        if layer == 0:
                            # fp16 staging + row sum of squares via ACT accum
                            ow = epool.tile([128, 128], f16, name=f"o16_{w}",
                                            tag=f"o16_{w}", bufs=1)
                            nc.vector.tensor_copy(out=ow[:], in_=o_ps[:])
                            o16.append(ow)
                            sqj = epool.tile([128, 128], f32, tag="sqj")
                            nc.scalar.activation(
                                out=sqj[:], in_=o_ps[:], func=Act.Square,
                                accum_out=ss_all[:, w:w + 1])
                        else:
                            o_sb = epool.tile([128, 128], f32, tag="osb")
                            nc.scalar.activation(out=o_sb[:], in_=o_ps[:],
                                                 func=Act.Copy)
                            nc.sync.dma_start(
                                out=t_out[w * 128:(w + 1) * 128, :], in_=o_sb[:])

                if layer == 0:
                    nrm_all = epool.tile([128, WPC], f32, name="nrm_all",
                                         tag="nrm_all", bufs=1)
                    nc.scalar.activation(out=nrm_all[:], in_=ss_all[:],
                                         func=Act.Sqrt, bias=eps_sb[:])
                    rn_all = epool.tile([128, WPC], f32, name="rn_all",
                                        tag="rn_all", bufs=1)
                    nc.vector.reciprocal(out=rn_all[:], in_=nrm_all[:])
                    for w in range(WPC):
                        h1_sb = epool.tile([128, 128], f16, tag="h1")
                        nc.vector.tensor_scalar(
                            out=h1_sb[:], in0=o16[w][:],
                            scalar1=rn_all[:, w:w + 1],
                            scalar2=zero_sb[:],
                            op0=Alu.mult, op1=Alu.max)
                        nc.sync.dma_start(
                            out=h1_my[w * 128:(w + 1) * 128, :], in_=h1_sb[:])
                    nc.gpsimd.collective_compute(
                        "AllGather",
                        mybir.AluOpType.bypass,
                        replica_groups=[list(range(n_cores))],
                        ins=[h1_my.opt()],
                        outs=[h1_all.opt()],
                    )

    if legalize:
        _legalize_sync_waits(nc)
    return nc


def kernel(**inputs):
    import sys
    if '/opt/trn_rl_repo' not in sys.path:
        sys.path.insert(0, '/opt/trn_rl_repo')

    prep = _prep(
        inputs["x"], inputs["W_self1"], inputs["W_neigh1"], inputs["b1"],
        inputs["W_self2"], inputs["W_neigh2"], inputs["b2"],
        inputs["edge_index"], inputs["edge_type"])
    nchunk, NCH = prep[-2], prep[-1]

    nc = build_module(NCH, nchunk, legalize=True, n_cores=C)

    from concourse.bass_utils import run_bass_kernel_spmd
    res = run_bass_kernel_spmd(nc, make_in_maps(prep), core_ids=list(range(C)))

    out = np.empty((N, F), dtype=np.float32)
    for c in range(C):
        out[c * NPC:(c + 1) * NPC] = res.results[c]["out"][:NPC]
    return out


# revision 5
# speedup vs baseline: 3.5055x; 1.0711x over previous
"""Trainium2 Bass kernel for nn_MetricalGNN (2-layer hetero GraphSAGE).

Math (per layer, T=4 edge types):
    out = h @ mean_t(W_self[t]) + mean_t(b[t])
        + (1/T) * sum_t diag(1/max(cnt_t,1)) @ segsum_t(h[src]) @ W_neigh[t]
Layer 1 is followed by row-wise L2 normalize + ReLU.

Device strategy (8 cores, destination-sharded):
  - Each core owns a contiguous 6250-node destination range, processed in
    49 windows of 128 destinations.
  - Per (core, window) the row stream is [self(128) | t0 | t1 | t2 | t3]
    packed TIGHTLY into 128-row chunks (chunk count = max over cores, the
    only padding).  Self rows are the window's own nodes with scale 1.
  - All chunks of a window PAIR are fetched by ONE batched indirect DMA
    (SWDGE desc-gen has ~1us fixed cost per instruction; batching
    amortizes it ~28x).
  - Chunks may straddle class boundaries: a static per-window sub-matmul
    schedule (union of classes present in each chunk slot over all cores)
    routes each class run to its own PSUM bank via a scaled one-hot
    A[e,d] = scale_e*(iota==dst_e) built in one fp16 DVE tensor_scalar
    (rows of other classes get dst=200 -> zero).  The scale folds in the
    1/cnt mean and the 1/T type average.  Chunk 0 of a full window is
    pure self and uses a shared constant identity matrix (no DVE op).
  - Stage 2 per window: copy the five S^T banks to SBUF (fp16) and run
    five matmuls out[d,fo] += S_cls @ W_cls into one PSUM bank, plus a
    K=1 ones-row matmul adding the mean bias.
  - Layer-1 epilogue: ACT Square+accum produces row sums of squares
    directly from PSUM; sqrt/reciprocal batched once per layer; DVE
    applies relu(h*rn) and h1 is stored as fp16.
  - An AllGather exchanges the per-core h1 slices between layers; layer 2
    gathers from the concatenated [8*6272, 128] buffer via remapped
    indices.

Inputs are replicated/sharded on the host: x is pre-cast to fp16 and
replicated; per-core metadata tensors carry gather indices (one column
per chunk) and one-hot dst/scale columns (one per sub-matmul, fp32);
weights are packed to fp16 once.
"""

import numpy as np

N = 50000
E = 600000
F = 128
T = 4
NCLS = T + 1               # class 0 = self, 1..4 = edge types
C = 8                      # cores
NPC = N // C               # 6250 destinations per core
WPC = (NPC + 127) // 128   # 49 windows per core
NPC_PAD = WPC * 128        # 6272 rows per core slice
GB = 2                     # windows per batched gather instruction
PAD_DST = 200.0            # one-hot miss -> zero column


def _win_groups():
    """Windows grouped GB at a time for batched gathers."""
    return [tuple(range(w, min(w + GB, WPC))) for w in range(0, WPC, GB)]


def _prep(x, W_self1, W_neigh1, b1, W_self2, W_neigh2, b2, edge_index, edge_type):
    src = np.asarray(edge_index[0], dtype=np.int64)
    dst = np.asarray(edge_index[1], dtype=np.int64)
    et = np.asarray(edge_type, dtype=np.int64)

    cnt = np.bincount(et * N + dst, minlength=T * N).reshape(T, N).astype(np.float32)
    scale_e = (0.25 / np.maximum(cnt[et, dst], 1.0)).astype(np.float32)

    # per-row records: edges (cls = 1+type) + self rows (cls 0, scale 1)
    e_core = dst // NPC
    e_win = (dst % NPC) // 128
    e_dloc = ((dst % NPC) % 128).astype(np.float32)
    e_cls = (1 + et).astype(np.int64)
    e_i1 = src.astype(np.int32)
    e_i2 = ((src // NPC) * NPC_PAD + (src % NPC)).astype(np.int32)

    s_node = np.arange(N, dtype=np.int64)
    s_core = s_node // NPC
    s_pos = s_node % NPC
    s_win = s_pos // 128
    s_dloc = (s_pos % 128).astype(np.float32)
    s_cls = np.zeros(N, dtype=np.int64)
    s_i1 = s_node.astype(np.int32)
    s_i2 = (s_core * NPC_PAD + s_pos).astype(np.int32)
    s_scale = np.ones(N, dtype=np.float32)

    r_core = np.concatenate([s_core, e_core])
    r_win = np.concatenate([s_win, e_win])
    r_cls = np.concatenate([s_cls, e_cls])
    r_dloc = np.concatenate([s_dloc, e_dloc]).astype(np.float32)
    r_scale = np.concatenate([s_scale, scale_e]).astype(np.float32)
    r_i1 = np.concatenate([s_i1, e_i1])
    r_i2 = np.concatenate([s_i2, e_i2])

    order = np.lexsort((r_cls, r_win, r_core))
    r_core, r_win, r_cls = r_core[order], r_win[order], r_cls[order]
    r_dloc, r_scale = r_dloc[order], r_scale[order]
    r_i1, r_i2 = r_i1[order], r_i2[order]

    cw = r_core * WPC + r_win
    n_cw = np.bincount(cw, minlength=C * WPC).reshape(C, WPC)
    lo_cw = np.zeros(C * WPC + 1, dtype=np.int64)
    np.cumsum(n_cw.reshape(-1), out=lo_cw[1:])

    K_w = -(-n_cw.max(axis=0) // 128)            # chunks per window [WPC]
    colbase = np.zeros(WPC, dtype=np.int64)
    colbase[1:] = np.cumsum(K_w)[:-1]
    NCHG = int(K_w.sum())

    # per (w, k): union over cores of classes present -> sub-matmul schedule
    # sched[w] = list of (k, cls, j, start, stop)
    present = np.zeros((WPC, int(K_w.max()), NCLS), dtype=bool)
    for c in range(C):
        for w in range(WPC):
            n = int(n_cw[c, w])
            lo = int(lo_cw[c * WPC + w])
            kk = np.arange(n) // 128
            present[w, kk, r_cls[lo:lo + n]] = True

    sched = []
    NSUB = 0
    for w in range(WPC):
        subs = []
        for k in range(int(K_w[w])):
            for cls in range(NCLS):
                if present[w, k, cls]:
                    subs.append([k, cls, NSUB, False, False])
                    NSUB += 1
        for cls in range(NCLS):
            own = [s for s in subs if s[1] == cls]
            if own:
                own[0][3] = True
                own[-1][4] = True
        sched.append([tuple(s) for s in subs])

    idx1 = np.zeros((C, NCHG, 128), dtype=np.int32)
    idx2 = np.zeros((C, NCHG, 128), dtype=np.int32)
    dstc = np.full((C, NSUB, 128), PAD_DST, dtype=np.float32)
    sclc = np.zeros((C, NSUB, 128), dtype=np.float32)

    for c in range(C):
        flat_i1 = idx1[c].reshape(-1)
        flat_i2 = idx2[c].reshape(-1)
        for w in range(WPC):
            n = int(n_cw[c, w])
            lo = int(lo_cw[c * WPC + w])
            s0 = int(colbase[w]) * 128
            flat_i1[s0:s0 + n] = r_i1[lo:lo + n]
            flat_i2[s0:s0 + n] = r_i2[lo:lo + n]
            kk = np.arange(n) // 128
            rr = np.arange(n) % 128
            cls_n = r_cls[lo:lo + n]
            for (k, cls, j, _st, _sp) in sched[w]:
                m = (kk == k) & (cls_n == cls)
                if m.any():
                    dstc[c, j, rr[m]] = r_dloc[lo:lo + n][m]
                    sclc[c, j, rr[m]] = r_scale[lo:lo + n][m]

    # -> [C, 128, NCHG]/[C, 128, NSUB] so column k holds chunk/sub k's rows
    idx1 = np.ascontiguousarray(idx1.transpose(0, 2, 1))
    idx2 = np.ascontiguousarray(idx2.transpose(0, 2, 1))
    dstc = np.ascontiguousarray(dstc.transpose(0, 2, 1))
    sclc = np.ascontiguousarray(sclc.transpose(0, 2, 1))

    # weight order matched to class ids: slot 0 = W_self_avg, 1..4 = W_neigh
    wpack = np.empty((2 * NCLS, F, F), dtype=np.float16)
    wpack[0] = np.asarray(W_self1, np.float32).mean(axis=0).astype(np.float16)
    wpack[1:NCLS] = np.asarray(W_neigh1, np.float32).astype(np.float16)
    wpack[NCLS] = np.asarray(W_self2, np.float32).mean(axis=0).astype(np.float16)
    wpack[NCLS + 1:] = np.asarray(W_neigh2, np.float32).astype(np.float16)

    bpack = np.stack([
        np.asarray(b1, np.float32).mean(axis=0),
        np.asarray(b2, np.float32).mean(axis=0),
    ]).astype(np.float16)

    x16 = np.asarray(x, np.float32).astype(np.float16)
    meta = {"K_w": K_w, "colbase": colbase, "NCHG": NCHG,
            "NSUB": NSUB, "sched": sched}
    return idx1, idx2, dstc, sclc, wpack, bpack, x16, meta


def make_in_maps(prep):
    idx1, idx2, dstc, sclc, wpack, bpack, x16, meta = prep
    return [
        {"x16": x16, "idx1": idx1[c], "idx2": idx2[c],
         "dstc": dstc[c], "sclc": sclc[c], "wpack": wpack, "bpack": bpack}
        for c in range(C)
    ]


def _legalize_sync_waits(nc, max_waits=1):
    """The walrus build in this container caps sync-wait commands per
    instruction; hoist excess waits onto NOPs inserted before the
    instruction on the same engine (sequencers execute in order)."""
    from concourse import mybir

    ctr = [0]
    for fn in nc.m.functions:
        for bb in fn.blocks:
            insts = bb.instructions
            if not any(
                i.sync_info is not None and len(i.sync_info.on_wait) > max_waits
                for i in insts
            ):
                continue
            out = []
            for inst in insts:
                si = inst.sync_info
                if si is not None and len(si.on_wait) > max_waits:
                    waits = list(si.on_wait)
                    keep = waits[-max_waits:]
                    hoist = waits[:-max_waits]
                    for i in range(0, len(hoist), max_waits):
                        nop = mybir.InstNoOp(
                            name=f"I-waitsplit-{ctr[0]}", ins=[], outs=[])
                        ctr[0] += 1
                        nop.engine = inst.engine
                        nop.sync_info = mybir.SyncInfo(
                            on_wait=hoist[i:i + max_waits], on_update=[])
                        out.append(nop)
                    inst.sync_info = mybir.SyncInfo(
                        on_wait=keep, on_update=list(si.on_update))
                out.append(inst)
            insts.clear()
            insts.extend(out)


def build_module(meta, legalize=True, n_cores=C):
    import concourse.bass as bass
    import concourse.tile as tile
    from concourse import mybir

    f16, f32, i32 = mybir.dt.float16, mybir.dt.float32, mybir.dt.int32
    Alu = mybir.AluOpType
    Act = mybir.ActivationFunctionType

    K_w, colbase = meta["K_w"], meta["colbase"]
    NCHG, NSUB, sched = meta["NCHG"], meta["NSUB"], meta["sched"]

    nc = bass.Bass(trn_type="TRN2")
    t_x16 = nc.dram_tensor("x16", [N, F], f16, kind="ExternalInput")
    t_idx1 = nc.dram_tensor("idx1", [128, NCHG], i32, kind="ExternalInput")
    t_idx2 = nc.dram_tensor("idx2", [128, NCHG], i32, kind="ExternalInput")
    t_dstc = nc.dram_tensor("dstc", [128, NSUB], f32, kind="ExternalInput")
    t_sclc = nc.dram_tensor("sclc", [128, NSUB], f32, kind="ExternalInput")
    t_wpack = nc.dram_tensor("wpack", [2 * NCLS, F, F], f16, kind="ExternalInput")
    t_bpack = nc.dram_tensor("bpack", [2, F], f16, kind="ExternalInput")
    t_out = nc.dram_tensor("out", [NPC_PAD, F], f32, kind="ExternalOutput")

    groups = _win_groups()
    maxcols = max(int(sum(K_w[w] for w in g)) for g in groups)

    with tile.TileContext(nc, num_cores=n_cores) as tc:
        with tc.tile_pool(name="const", bufs=1) as cpool, \
             tc.tile_pool(name="gath", bufs=2) as gpool, \
             tc.tile_pool(name="onehot", bufs=8) as apool, \
             tc.tile_pool(name="stage2", bufs=2) as spool, \
             tc.tile_pool(name="epi", bufs=2) as epool, \
             tc.tile_pool(name="spsum", bufs=1, space="PSUM") as pspool, \
             tc.tile_pool(name="opsum", bufs=2, space="PSUM") as opool, \
             tc.tile_pool(name="dram", bufs=1, space="DRAM") as dpool:

            idx1_t = cpool.tile([128, NCHG], i32)
            nc.sync.dma_start(out=idx1_t[:], in_=t_idx1[:])
            idx2_t = cpool.tile([128, NCHG], i32)
            nc.sync.dma_start(out=idx2_t[:], in_=t_idx2[:])
            dstc_t = cpool.tile([128, NSUB], f32)
            nc.sync.dma_start(out=dstc_t[:], in_=t_dstc[:])
            sclc_t = cpool.tile([128, NSUB], f32)
            nc.sync.dma_start(out=sclc_t[:], in_=t_sclc[:])

            w_sb = cpool.tile([128, 2 * NCLS * F], f16)
            for k in range(2 * NCLS):
                nc.sync.dma_start(out=w_sb[:, k * F:(k + 1) * F], in_=t_wpack[k])
            b_sb = cpool.tile([1, 2 * F], f16)
            nc.sync.dma_start(out=b_sb[:, :F], in_=t_bpack[0:1, :])
            nc.sync.dma_start(out=b_sb[:, F:], in_=t_bpack[1:2, :])
            ones_sb = cpool.tile([1, 128], f16)
            nc.vector.memset(ones_sb[:], 1.0)
            eps_sb = cpool.tile([128, 1], f32)
            nc.vector.memset(eps_sb[:], 1e-24)
            zero_sb = cpool.tile([128, 1], f32)
            nc.vector.memset(zero_sb[:], 0.0)

            iota_i = cpool.tile([128, 128], i32)
            nc.gpsimd.iota(iota_i[:], pattern=[[1, 128]], base=0, channel_multiplier=0)
            iota16 = cpool.tile([128, 128], f16)
            nc.vector.tensor_copy(out=iota16[:], in_=iota_i[:])
            iotap_i = cpool.tile([128, 1], i32)
            nc.gpsimd.iota(iotap_i[:], pattern=[[0, 1]], base=0, channel_multiplier=1)
            iotap32 = cpool.tile([128, 1], f32)
            nc.vector.tensor_copy(out=iotap32[:], in_=iotap_i[:])
            # shared identity one-hot for full self chunks
            ident = cpool.tile([128, 128], f16)
            nc.vector.tensor_scalar(
                out=ident[:], in0=iota16[:], scalar1=iotap32[:],
                scalar2=None, op0=Alu.is_equal)

            h1_my = dpool.tile([NPC_PAD, F], f16)
            h1_all = dpool.tile([C * NPC_PAD, F], f16, addr_space="Shared")

            for layer in (0, 1):
                src_tbl = t_x16 if layer == 0 else h1_all
                idx_t = idx1_t if layer == 0 else idx2_t
                wofs = layer * NCLS * F

                ss_all = epool.tile([128, WPC], f32, name=f"ss_all{layer}",
                                    tag=f"ss_all{layer}", bufs=1)
                o16 = []

                for grp in groups:
                    col0 = int(colbase[grp[0]])
                    cols = int(sum(K_w[w] for w in grp))
                    m_t = gpool.tile([128, maxcols * F], f16, tag="m")
                    nc.gpsimd.indirect_dma_start(
                        out=m_t[:, :cols * F], out_offset=None, in_=src_tbl[:],
                        in_offset=bass.IndirectOffsetOnAxis(
                            ap=idx_t[:, col0:col0 + cols], axis=0))

                    for w in grp:
                        s_ps = [pspool.tile([128, 128], f32, space="PSUM",
                                            name=f"s{t}", tag=f"s{t}")
                                for t in range(NCLS)]
                        wcol0 = int(colbase[w])
                        for (k, cls, j, st, sp) in sched[w]:
                            if cls == 0 and w < WPC - 1:
                                a_t = ident
                            else:
                                a_t = apool.tile([128, 128], f16, tag="a")
                                nc.vector.tensor_scalar(
                                    out=a_t[:], in0=iota16[:],
                                    scalar1=dstc_t[:, j:j + 1],
                                    scalar2=sclc_t[:, j:j + 1],
                                    op0=Alu.is_equal, op1=Alu.mult)
                            off = (wcol0 - col0 + k) * F
                            nc.tensor.matmul(
                                out=s_ps[cls][:], lhsT=m_t[:, off:off + F],
                                rhs=a_t[:], start=st, stop=sp)

                        # stage 2
                        o_ps = opool.tile([128, 128], f32, space="PSUM", tag="o")
                        s_sb = []
                        for t in range(NCLS):
                            stile = spool.tile([128, 128], f16, tag=f"ssb{t}",
                                               name=f"ssb{t}")
                            if t < 2:
                                nc.vector.tensor_copy(out=stile[:], in_=s_ps[t][:])
                            else:
                                nc.scalar.activation(out=stile[:], in_=s_ps[t][:],
                                                     func=Act.Copy)
                            s_sb.append(stile)
                        for t in range(NCLS):
                            nc.tensor.matmul(
                                out=o_ps[:], lhsT=s_sb[t][:],
                                rhs=w_sb[:, wofs + t * F: wofs + (t + 1) * F],
                                start=(t == 0), stop=False)
                        nc.tensor.matmul(
                            out=o_ps[:], lhsT=ones_sb[:],
                            rhs=b_sb[:, layer * F:(layer + 1) * F],
                            start=False, stop=True)

                        if layer == 0:
                            # fp16 staging + row sum of squares via ACT accum
                            ow = epool.tile([128, 128], f16, name=f"o16_{w}",
                                            tag=f"o16_{w}", bufs=1)
                            nc.vector.tensor_copy(out=ow[:], in_=o_ps[:])
                            o16.append(ow)
                            sqj = epool.tile([128, 128], f32, tag="sqj")
                            nc.scalar.activation(
                                out=sqj[:], in_=o_ps[:], func=Act.Square,
                                accum_out=ss_all[:, w:w + 1])
                        else:
                            o_sb = epool.tile([128, 128], f32, tag="osb")
                            nc.scalar.activation(out=o_sb[:], in_=o_ps[:],
                                                 func=Act.Copy)
                            nc.sync.dma_start(
                                out=t_out[w * 128:(w + 1) * 128, :], in_=o_sb[:])

                if layer == 0:
                    nrm_all = epool.tile([128, WPC], f32, name="nrm_all",
                                         tag="nrm_all", bufs=1)
                    nc.scalar.activation(out=nrm_all[:], in_=ss_all[:],
                                         func=Act.Sqrt, bias=eps_sb[:])
                    rn_all = epool.tile([128, WPC], f32, name="rn_all",
                                        tag="rn_all", bufs=1)
                    nc.vector.reciprocal(out=rn_all[:], in_=nrm_all[:])
                    for w in range(WPC):
                        h1_sb = epool.tile([128, 128], f16, tag="h1")
                        nc.vector.tensor_scalar(
                            out=h1_sb[:], in0=o16[w][:],
                            scalar1=rn_all[:, w:w + 1],
                            scalar2=zero_sb[:],
                            op0=Alu.mult, op1=Alu.max)
                        nc.sync.dma_start(
                            out=h1_my[w * 128:(w + 1) * 128, :], in_=h1_sb[:])
                    nc.gpsimd.collective_compute(
                        "AllGather",
                        mybir.AluOpType.bypass,
                        replica_groups=[list(range(n_cores))],
                        ins=[h1_my.opt()],
                        outs=[h1_all.opt()],
                    )

    if legalize:
        _legalize_sync_waits(nc)
    return nc


def kernel(**inputs):
    import sys
    if '/opt/trn_rl_repo' not in sys.path:
        sys.path.insert(0, '/opt/trn_rl_repo')

    prep = _prep(
        inputs["x"], inputs["W_self1"], inputs["W_neigh1"], inputs["b1"],
        inputs["W_self2"], inputs["W_neigh2"], inputs["b2"],
        inputs["edge_index"], inputs["edge_type"])

    nc = build_module(prep[-1], legalize=True, n_cores=C)

    from concourse.bass_utils import run_bass_kernel_spmd
    res = run_bass_kernel_spmd(nc, make_in_maps(prep), core_ids=list(range(C)))

    out = np.empty((N, F), dtype=np.float32)
    for c in range(C):
        out[c * NPC:(c + 1) * NPC] = res.results[c]["out"][:NPC]
    return out
